# revision 44
# baseline (speedup 1.0000x reference)
"""Trainium2 Bass kernel for nn_ClusterEncoder (PointTransformerConv-style
GNN message passing), 8-core SPMD.

The axon-tunneled host<->device wire (~45 MB/s up, ~29 MB/s down) dominates
wall time, so the design minimizes shipped bytes end to end:
  * x ships int8 with per-row scales (6.4 MB total instead of 8x25.6 MB
    replicated f32); pos ships sharded f32; per-edge payload is just
    srcid (u16) + dstloc (u8) + per-chunk aux (u16) -- posd and dst ids are
    derived on device; y returns fp16.  A persistent XLA compilation cache
    makes repeat invocations skip the HLO->NEFF pipeline.

Strategy (edges sharded by destination node):
  * Host: sort edges by dst, split nodes into 8 equal contiguous ranges
    (edge counts balance to ~0.3% for this random graph). Within a core,
    pack CONTIGUOUS node windows [n0, n0+cnt) into "chunks" of <=128 nodes
    and <=CHUNK_E edges; pad each chunk's edge list to CHUNK_E slots.
  * Device, phase 1 (x sharded by node range): dequantize x, then
    U = x_loc @ (W_dst@Wa1) -> local DRAM [NLOC, 64+2] with pos appended
    (dst rows are core-local by construction), VH_loc = x_loc @
    [W_src@Wa1 | W_lin] with pos appended -> AllGather over NeuronLink
    into full VH [N, 194] for src gathers.
  * Device, phase 2 (per chunk of 16 x 128-edge tiles):
      - dst = min(dstloc + n0, NLOC-1) (pad lanes clamp to a valid row and
        contribute zero through the zero indicator row),
      - gather VH rows by global src and U rows by local dst,
      - posd = pos[dst] - pos[src] from the gathered columns, transposed
        into a [2, CHUNK_E] tile for the pos MLP,
      - gd = U[dst] - V[src]  (attn-layer-1 folded through node features),
      - pos MLP: t_p1 = relu(Wp1^T posd^T + bp1), delta = relu(Wp2^T t_p1 + bp2),
      - z1 = Wa1^T delta;  t_a = relu(z1 + gd^T + ba1),
      - logits = relu(Wa2^T t_a + ba2);  e = exp(logits - SHIFT)
        (softmax max-subtraction replaced by a constant shift -- exactly
        equivalent math since the shift cancels in e/sum(e); logits are
        relu-bounded so no overflow),
      - one-hot indicator per tile from local dst index (iota + is_equal),
      - segment-sum via matmul: acc[n, 0:128] += ind^T @ (e*(H[src]+delta))^T,
        acc[n, 128:256] += ind^T @ e^T   (numerator and normalizer together),
      - out = relu(NUM / (s + eps)); indirect-scatter fp16 rows to y.
  * Softmax segments are core-local by construction; the only collective is
    the phase-1 AllGather.
"""
import sys
from dataclasses import dataclass
from math import ceil

if "/opt/trn_rl_repo" not in sys.path:
    sys.path.insert(0, "/opt/trn_rl_repo")

import ml_dtypes
import numpy as np
import jax

# Persistent XLA compilation cache: repeated kernel invocations (fresh
# jax.jit closures inside run_bass_kernel_spmd) hit the disk cache instead
# of re-running the HLO pipeline + BIR->NEFF hook (~0.8s/call -> ~0.04s).
jax.config.update("jax_compilation_cache_dir", "/tmp/jax_comp_cache")
jax.config.update("jax_persistent_cache_min_compile_time_secs", 0)
try:
    jax.config.update("jax_persistent_cache_min_entry_size_bytes", 0)
except Exception:
    pass

import concourse.bass as bass
import concourse.mybir as mybir
import concourse.tile as tile
from concourse import bacc
from concourse.bass import IndirectOffsetOnAxis
from concourse.bass_utils import run_bass_kernel_spmd
from concourse.masks import make_identity

f32 = mybir.dt.float32
f32r = mybir.dt.float32r
bf16 = mybir.dt.bfloat16
f16 = mybir.dt.float16
i32 = mybir.dt.int32
i8 = mybir.dt.int8
u16 = mybir.dt.uint16
u8 = mybir.dt.uint8
AF = mybir.ActivationFunctionType
ALU = mybir.AluOpType


@dataclass
class Cfg:
    N: int = 50000
    C: int = 128
    PH: int = 64
    AH: int = 64
    DIM: int = 2
    M: int = 8            # cores
    T: int = 16           # 128-edge tiles per chunk
    TB: int = 4           # tiles per matmul block (block = 512 edges)
    SHIFT: float = 8.0
    EPS: float = 1e-12
    mm_dt: object = f32r  # matmul compute dtype (f32r: 1 cyc/row at free>=256)

    @property
    def NLOC(self):
        return self.N // self.M

    @property
    def CHUNK_E(self):
        return self.T * 128

    @property
    def OUT_ROWS(self):
        return self.NLOC + 1  # +1 trash row for padded scatter lanes


CFG = Cfg()


# ---------------------------------------------------------------- host pack
def _pack(x, pos, edge_index, cfg):
    """Sort/shard/chunk edges; returns per-core input dicts (minus weights).

    Chunks are CONTIGUOUS local-node windows [n0, n0+cnt) with cnt<=128 and
    total edge count <=CHUNK_E (isolated nodes just occupy a row and output
    zero).  Contiguity lets the device derive the gather ids:
    dst = min(dstloc + n0, NLOC-1), so only dstloc (u8) + aux (u16 outrow/n0)
    ship per edge tile.  posd is computed on device from pos columns carried
    in the gathered U/VH rows, so no per-edge pos payload ships at all.
    """
    src = np.asarray(edge_index[0], np.int64)
    dst = np.asarray(edge_index[1], np.int64)
    order = np.argsort(dst, kind="stable")
    s_s = src[order]
    d_s = dst[order]

    NLOC = cfg.NLOC
    bounds = np.searchsorted(d_s, np.arange(cfg.M + 1) * NLOC)

    cores = []
    for c in range(cfg.M):
        lo, hi = bounds[c], bounds[c + 1]
        dloc = d_s[lo:hi] - c * NLOC
        deg = np.bincount(dloc, minlength=NLOC)
        chunks = []  # (n0, cnt, e0, e1) ; e relative to lo
        n0, e0 = 0, 0
        while n0 < NLOC:
            cnt, cur_e = 0, 0
            while (n0 + cnt < NLOC and cnt < 128
                   and cur_e + deg[n0 + cnt] <= cfg.CHUNK_E):
                cur_e += int(deg[n0 + cnt])
                cnt += 1
            assert cnt > 0, f"degree {deg[n0]} exceeds chunk capacity"
            chunks.append((n0, cnt, e0, e0 + cur_e))
            n0 += cnt
            e0 += cur_e
        cores.append((lo, chunks, dloc))

    NCHUNK = max(len(ch) for _, ch, _ in cores)

    in_maps = []
    for c in range(cfg.M):
        lo, chunks, dloc = cores[c]
        srcid = np.zeros((NCHUNK, 128, cfg.T), np.uint16)
        dstloc = np.full((NCHUNK, 128, cfg.T), 255, np.uint8)  # 255 = pad lane
        aux = np.zeros((NCHUNK, 128, 2), np.uint16)
        aux[:, :, 0] = cfg.NLOC  # default scatter row = trash
        for k, (n0, cnt, e0, e1) in enumerate(chunks):
            ecnt = e1 - e0
            g0, g1 = lo + e0, lo + e1
            j = np.arange(ecnt)
            t_idx = j >> 7
            lane = j & 127
            srcid[k, lane, t_idx] = s_s[g0:g1].astype(np.uint16)
            dstloc[k, lane, t_idx] = (dloc[e0:e1] - n0).astype(np.uint8)
            aux[k, :cnt, 0] = (n0 + np.arange(cnt)).astype(np.uint16)
            aux[k, :, 1] = n0
        in_maps.append(dict(srcid=srcid, dstloc=dstloc, aux=aux))
    return in_maps, NCHUNK


# ---------------------------------------------------------------- program
def _build(cfg, nchunk):
    nc = bacc.Bacc(None, target_bir_lowering=False)
    N, C, PH, AH, DIM = cfg.N, cfg.C, cfg.PH, cfg.AH, cfg.DIM
    NLOC = cfg.NLOC
    mdt = cfg.mm_dt

    x_d = nc.declare_dram_parameter("x", [NLOC, C], i8, isOutput=False)
    xs_d = nc.declare_dram_parameter("xscale", [NLOC, 1], f32, isOutput=False)
    pos_d = nc.declare_dram_parameter("pos", [NLOC, DIM], f32, isOutput=False)
    wnode_d = nc.declare_dram_parameter("Wnode", [C, 2 * AH + C], f32, isOutput=False)
    wp1_d = nc.declare_dram_parameter("Wp1", [DIM, PH], f32, isOutput=False)
    wp2_d = nc.declare_dram_parameter("Wp2", [PH, C], f32, isOutput=False)
    wa1_d = nc.declare_dram_parameter("Wa1p", [C, AH], f32, isOutput=False)
    wa2_d = nc.declare_dram_parameter("Wa2", [AH, C], f32, isOutput=False)
    bias_d = nc.declare_dram_parameter("bias", [128, 5], f32, isOutput=False)
    src_d = nc.declare_dram_parameter("srcid", [nchunk, 128, cfg.T], u16, isOutput=False)
    dl_d = nc.declare_dram_parameter("dstloc", [nchunk, 128, cfg.T], u8, isOutput=False)
    aux_d = nc.declare_dram_parameter("aux", [nchunk, 128, 2], u16, isOutput=False)
    y_d = nc.declare_dram_parameter("y", [cfg.OUT_ROWS, C], f16, isOutput=True)

    # U rows: [U(AH) | pos(DIM)] ; VH rows: [V(AH) | H(C) | pos(DIM)]
    UW = AH + DIM
    VW = AH + C + DIM
    U_d = nc.dram_tensor("U", [NLOC, UW], f32)         # x_loc @ (W_dst@Wa1)
    VHloc_d = nc.dram_tensor("VHl", [NLOC, VW], f32)
    VH_d = nc.dram_tensor("VH", [N, VW], f32, addr_space="Shared")

    NB = cfg.T // cfg.TB  # blocks per chunk
    BLK = cfg.TB * 128

    with tile.TileContext(nc) as tc:
        with tc.tile_pool(name="const", bufs=1) as cp:
            wnode_s = cp.tile([C, 2 * AH + C], f32)
            nc.sync.dma_start(out=wnode_s[:], in_=wnode_d[:, :])
            wp1_s = cp.tile([DIM, PH], f32)
            nc.sync.dma_start(out=wp1_s[:], in_=wp1_d[:, :])
            wp2_s = cp.tile([PH, C], f32)
            nc.sync.dma_start(out=wp2_s[:], in_=wp2_d[:, :])
            wa2_s = cp.tile([AH, C], f32)
            nc.sync.dma_start(out=wa2_s[:], in_=wa2_d[:, :])
            bias_s = cp.tile([128, 5], f32)
            nc.sync.dma_start(out=bias_s[:], in_=bias_d[:, :])
            ident_s = cp.tile([128, 128], f32)
            make_identity(nc, ident_s[:])
            iota_i = cp.tile([128, 128], i32)
            nc.gpsimd.iota(iota_i[:], pattern=[[1, 128]], base=0, channel_multiplier=0)
            iota_s = cp.tile([128, 128], f32)
            nc.vector.tensor_copy(iota_s[:], iota_i[:])
            wa1_s = cp.tile([C, AH], f32)
            nc.sync.dma_start(out=wa1_s[:], in_=wa1_d[:, :])

            # fp32r matmul operands must be produced rounded-to-f32r: make
            # rounded copies of the stationary weights once.
            if mdt is f32r:
                wnode_m = cp.tile([C, 2 * AH + C], f32r)
                nc.vector.tensor_copy(wnode_m[:], wnode_s[:])
                wp1_m = cp.tile([DIM, PH], f32r)
                nc.vector.tensor_copy(wp1_m[:], wp1_s[:])
                wp2_m = cp.tile([PH, C], f32r)
                nc.vector.tensor_copy(wp2_m[:], wp2_s[:])
                wa1_m = cp.tile([C, AH], f32r)
                nc.vector.tensor_copy(wa1_m[:], wa1_s[:])
                wa2_m = cp.tile([AH, C], f32r)
                nc.vector.tensor_copy(wa2_m[:], wa2_s[:])
            else:
                wnode_m, wp1_m, wp2_m, wa1_m, wa2_m = wnode_s, wp1_s, wp2_s, wa1_s, wa2_s

            # ------- phase 1: local node features U / VH_loc, then AllGather
            with tc.tile_pool(name="p1", bufs=3) as p1, \
                 tc.tile_pool(name="p1ps", bufs=2, space="PSUM") as p1ps:
                nt = ceil(NLOC / 128)
                for t in range(nt):
                    r0 = t * 128
                    rows = min(128, NLOC - r0)
                    xq_t = p1.tile([128, C], i8, tag="xq")
                    nc.sync.dma_start(out=xq_t[:rows], in_=x_d[r0:r0 + rows, :])
                    xsc_t = p1.tile([128, 1], f32, tag="xsc")
                    nc.sync.dma_start(out=xsc_t[:rows], in_=xs_d[r0:r0 + rows, :])
                    xt = p1.tile([128, C], f32, tag="xt")
                    nc.vector.tensor_scalar(xt[:rows], xq_t[:rows],
                                            xsc_t[:rows, 0:1], None, op0=ALU.mult)
                    xT_p = p1ps.tile([128, 128], f32, tag="xT")
                    nc.tensor.transpose(xT_p[:, :rows], xt[:rows, :], ident_s[:rows, :rows])
                    xT_s = p1.tile([128, 128], mdt, tag="xTs")
                    nc.vector.tensor_copy(xT_s[:, :rows], xT_p[:, :rows])
                    uvh_p = p1ps.tile([128, 2 * AH + C], f32, tag="uvh")
                    nc.tensor.matmul(uvh_p[:rows, :], lhsT=xT_s[:, :rows],
                                     rhs=wnode_m[:], start=True, stop=True)
                    uvh_s = p1.tile([128, 2 * AH + C], f32, tag="uvhs")
                    nc.scalar.activation(uvh_s[:rows, :], uvh_p[:rows, :], AF.Copy)
                    post = p1.tile([128, DIM], f32, tag="post")
                    nc.sync.dma_start(out=post[:rows], in_=pos_d[r0:r0 + rows, :])
                    nc.sync.dma_start(out=U_d[r0:r0 + rows, 0:AH], in_=uvh_s[:rows, 0:AH])
                    nc.sync.dma_start(out=U_d[r0:r0 + rows, AH:UW], in_=post[:rows])
                    nc.sync.dma_start(out=VHloc_d[r0:r0 + rows, 0:AH + C], in_=uvh_s[:rows, AH:])
                    nc.sync.dma_start(out=VHloc_d[r0:r0 + rows, AH + C:VW], in_=post[:rows])

            nc.gpsimd.collective_compute(
                "AllGather",
                mybir.AluOpType.bypass,
                replica_groups=[list(range(cfg.M))],
                ins=[VHloc_d[:, :]],
                outs=[VH_d[:, :]],
            )

            # ---------------- phase 2: edges ----------------
            with tc.tile_pool(name="eb", bufs=3) as eb, \
                 tc.tile_pool(name="ebg", bufs=3) as ebg, \
                 tc.tile_pool(name="ps_acc", bufs=2, space="PSUM") as ps_acc, \
                 tc.tile_pool(name="ps_b", bufs=1, space="PSUM") as ps_b, \
                 tc.tile_pool(name="ps_c", bufs=1, space="PSUM") as ps_c, \
                 tc.tile_pool(name="ps_m", bufs=1, space="PSUM") as ps_m, \
                 tc.tile_pool(name="ps_n", bufs=1, space="PSUM") as ps_n, \
                 tc.tile_pool(name="ps_t", bufs=2, space="PSUM") as ps_t:
                for k in range(nchunk):
                    src16_s = eb.tile([128, cfg.T], u16, tag="src16")
                    nc.sync.dma_start(out=src16_s[:], in_=src_d[k, :, :])
                    src_s = eb.tile([128, cfg.T], i32, tag="src")
                    nc.vector.tensor_copy(src_s[:], src16_s[:])
                    dl8_s = eb.tile([128, cfg.T], u8, tag="dl8")
                    nc.sync.dma_start(out=dl8_s[:], in_=dl_d[k, :, :])
                    dl_s = eb.tile([128, cfg.T], f32, tag="dl")
                    nc.vector.tensor_copy(dl_s[:], dl8_s[:])
                    aux16_s = eb.tile([128, 2], u16, tag="aux16")
                    nc.sync.dma_start(out=aux16_s[:], in_=aux_d[k, :, :])
                    aux_s = eb.tile([128, 2], i32, tag="aux")
                    nc.vector.tensor_copy(aux_s[:], aux16_s[:])
                    aux_f = eb.tile([128, 2], f32, tag="auxf")
                    nc.vector.tensor_copy(aux_f[:], aux16_s[:])
                    # dst = min(dstloc + n0, NLOC-1): pad lanes (255) clamp to a
                    # valid row, contributing 0 through the zero indicator row.
                    dstf_s = eb.tile([128, cfg.T], f32, tag="dstf")
                    nc.vector.tensor_scalar(dstf_s[:], dl_s[:], aux_f[:, 1:2],
                                            float(NLOC - 1),
                                            op0=ALU.add, op1=ALU.min)
                    dst_s = eb.tile([128, cfg.T], i32, tag="dst")
                    nc.vector.tensor_copy(dst_s[:], dstf_s[:])
                    or_s = aux_s  # column 0 = scatter rows
                    pd_s = eb.tile([DIM, cfg.CHUNK_E], mdt, tag="pd")

                    acc_p = ps_acc.tile([128, 2 * C], f32, tag="acc")

                    for b in range(NB):
                        esl = slice(b * BLK, (b + 1) * BLK)
                        # gathers for this block, one [128,1]-offset DMA per tile
                        vhg_b = ebg.tile([128, cfg.TB, VW], f32, tag="vhgb")
                        ug_b = ebg.tile([128, cfg.TB, UW], f32, tag="ugb")
                        for tt in range(cfg.TB):
                            ti = b * cfg.TB + tt
                            nc.gpsimd.indirect_dma_start(
                                out=vhg_b[:, tt, :], out_offset=None, in_=VH_d[:],
                                in_offset=IndirectOffsetOnAxis(
                                    ap=src_s[:, ti:ti + 1], axis=0))
                            nc.gpsimd.indirect_dma_start(
                                out=ug_b[:, tt, :], out_offset=None, in_=U_d[:],
                                in_offset=IndirectOffsetOnAxis(
                                    ap=dst_s[:, ti:ti + 1], axis=0))
                        vhgs = [vhg_b[:, tt, :] for tt in range(cfg.TB)]
                        ugs = [ug_b[:, tt, :] for tt in range(cfg.TB)]

                        # posd = pos[dst] - pos[src] from gathered columns;
                        # transpose [128,2] -> [2,128] into the chunk pd tile
                        posd_s = eb.tile([128, cfg.TB, DIM], f32, tag="posd")
                        nc.vector.tensor_tensor(
                            posd_s[:, :, :], ug_b[:, :, AH:UW],
                            vhg_b[:, :, AH + C:VW], op=ALU.subtract)
                        for tt in range(cfg.TB):
                            ti = b * cfg.TB + tt
                            pdT_p = ps_t.tile([128, 128], f32, tag="tr")
                            nc.tensor.transpose(pdT_p[:DIM, :], posd_s[:, tt, :],
                                                ident_s[:])
                            nc.scalar.activation(
                                pd_s[:, ti * 128:(ti + 1) * 128],
                                pdT_p[:DIM, :], AF.Copy)

                        # pos MLP
                        tp1_p = ps_m.tile([PH, BLK], f32, tag="tp1")
                        nc.tensor.matmul(tp1_p[:], lhsT=wp1_m[:],
                                         rhs=pd_s[:, esl], start=True, stop=True)
                        tp1_s = eb.tile([PH, BLK], mdt, tag="tp1s")
                        nc.scalar.activation(tp1_s[:], tp1_p[:], AF.Relu, bias=bias_s[0:PH, 0:1])
                        del_p = ps_b.tile([C, BLK], f32, tag="delp")
                        nc.tensor.matmul(del_p[:], lhsT=wp2_m[:],
                                         rhs=tp1_s[:], start=True, stop=True)
                        del_s = eb.tile([C, BLK], f32, tag="dels")
                        nc.scalar.activation(del_s[:], del_p[:], AF.Relu, bias=bias_s[:, 1:2])
                        if mdt is f32r:
                            del_m = eb.tile([C, BLK], f32r, tag="delm")
                            nc.scalar.activation(del_m[:], del_p[:], AF.Relu, bias=bias_s[:, 1:2])
                        else:
                            del_m = del_s

                        # attn layer 1: z1 = Wa1^T delta ; t_a = relu(z1 + gd^T + ba1)
                        z1_p = ps_n.tile([AH, BLK], f32, tag="z1")
                        nc.tensor.matmul(z1_p[:], lhsT=wa1_m[:],
                                         rhs=del_m[:], start=True, stop=True)
                        gd_b = eb.tile([128, cfg.TB, AH], f32, tag="gd")
                        nc.vector.tensor_tensor(gd_b[:, :, :], ug_b[:, :, 0:AH],
                                                vhg_b[:, :, 0:AH],
                                                op=ALU.subtract)
                        gdT_s = eb.tile([AH, BLK], f32, tag="gdT")
                        for tt in range(cfg.TB):
                            gdT_p = ps_t.tile([128, 128], f32, tag="tr")
                            nc.tensor.transpose(gdT_p[:AH, :], gd_b[:, tt, :],
                                                ident_s[:])
                            csl = slice(tt * 128, (tt + 1) * 128)
                            nc.scalar.activation(gdT_s[:, csl], gdT_p[:AH, :], AF.Copy)
                        tsum_s = eb.tile([AH, BLK], f32, tag="tsum")
                        nc.vector.tensor_tensor(tsum_s[:], z1_p[:], gdT_s[:],
                                                op=ALU.add)
                        ta_s = eb.tile([AH, BLK], mdt, tag="ta")
                        nc.scalar.activation(ta_s[:], tsum_s[:], AF.Relu, bias=bias_s[0:AH, 2:3])

                        # attn layer 2 + exp
                        al_p = ps_c.tile([C, BLK], f32, tag="al")
                        nc.tensor.matmul(al_p[:], lhsT=wa2_m[:],
                                         rhs=ta_s[:], start=True, stop=True)
                        ar_s = eb.tile([C, BLK], f32, tag="ar")
                        nc.scalar.activation(ar_s[:], al_p[:], AF.Relu, bias=bias_s[:, 3:4])
                        e_s = eb.tile([C, BLK], f32, tag="e")
                        nc.scalar.activation(e_s[:], ar_s[:], AF.Exp, bias=bias_s[:, 4:5])
                        ew2_s = eb.tile([C, BLK], f32, tag="ew2")
                        nc.vector.tensor_tensor(ew2_s[:], e_s[:], del_s[:], op=ALU.mult)
                        del del_s  # f32 copy only feeds ew2

                        # per-tile: transpose, assemble [ew | e]^T, indicator, seg-matmul
                        for tt in range(cfg.TB):
                            ti = b * cfg.TB + tt
                            csl = slice(tt * 128, (tt + 1) * 128)
                            eT_p = ps_t.tile([128, 128], f32, tag="tr")
                            nc.tensor.transpose(eT_p[:], e_s[:, csl], ident_s[:])
                            ew2T_p = ps_t.tile([128, 128], f32, tag="tr")
                            nc.tensor.transpose(ew2T_p[:], ew2_s[:, csl], ident_s[:])
                            ewe_s = eb.tile([128, 2 * C], mdt, tag="ewe")
                            nc.vector.tensor_copy(ewe_s[:, C:], eT_p[:])
                            tmp_s = eb.tile([128, C], f32, tag="tmp")
                            nc.vector.tensor_tensor(tmp_s[:], eT_p[:],
                                                    vhgs[tt][:, AH:AH + C],
                                                    op=ALU.mult)
                            nc.vector.tensor_tensor(ewe_s[:, 0:C], tmp_s[:], ew2T_p[:],
                                                    op=ALU.add)
                            ind_s = eb.tile([128, 128], mdt, tag="ind")
                            nc.vector.tensor_scalar(ind_s[:], iota_s[:], dl_s[:, ti:ti + 1],
                                                    None, op0=ALU.is_equal)
                            nc.tensor.matmul(acc_p[:], lhsT=ind_s[:],
                                             rhs=ewe_s[:],
                                             start=(ti == 0), stop=(ti == cfg.T - 1))

                    # finalize chunk
                    sp_s = eb.tile([128, C], f32, tag="sp")
                    nc.vector.tensor_scalar_add(sp_s[:], acc_p[:, C:], cfg.EPS)
                    rp_s = eb.tile([128, C], f32, tag="rp")
                    nc.vector.reciprocal(rp_s[:], sp_s[:])
                    o_s = eb.tile([128, C], f32, tag="o")
                    nc.vector.tensor_tensor(o_s[:], acc_p[:, 0:C], rp_s[:], op=ALU.mult)
                    o2_s = eb.tile([128, C], f16, tag="o2")
                    nc.scalar.activation(o2_s[:], o_s[:], AF.Relu)
                    nc.gpsimd.indirect_dma_start(
                        out=y_d[:], out_offset=IndirectOffsetOnAxis(ap=or_s[:, :1], axis=0),
                        in_=o2_s[:], in_offset=None)
    nc.finalize()
    return nc


def _build_inputs(inputs, cfg):
    x = np.ascontiguousarray(np.asarray(inputs["x"], np.float32))
    pos = np.ascontiguousarray(np.asarray(inputs["pos"], np.float32))
    W_lin = np.asarray(inputs["W_lin"], np.float32)
    W_src = np.asarray(inputs["W_src"], np.float32)
    W_dst = np.asarray(inputs["W_dst"], np.float32)
    Wp1 = np.asarray(inputs["Wp1"], np.float32)
    bp1 = np.asarray(inputs["bp1"], np.float32)
    Wp2 = np.asarray(inputs["Wp2"], np.float32)
    bp2 = np.asarray(inputs["bp2"], np.float32)
    Wa1 = np.asarray(inputs["Wa1"], np.float32)
    ba1 = np.asarray(inputs["ba1"], np.float32)
    Wa2 = np.asarray(inputs["Wa2"], np.float32)
    ba2 = np.asarray(inputs["ba2"], np.float32)

    Wda = (W_dst @ Wa1).astype(np.float32)   # [C, AH]
    Wsa = (W_src @ Wa1).astype(np.float32)
    wnode = np.concatenate([Wda, Wsa, W_lin], axis=1)  # [C, 2AH + C]
    bias = np.zeros((128, 5), np.float32)
    bias[: cfg.PH, 0] = bp1
    bias[: cfg.C, 1] = bp2
    bias[: cfg.AH, 2] = ba1
    bias[: cfg.C, 3] = ba2
    bias[:, 4] = -cfg.SHIFT

    packs, nchunk = _pack(x, pos, inputs["edge_index"], cfg)
    common = dict(Wnode=np.ascontiguousarray(wnode),
                  Wp1=np.ascontiguousarray(Wp1), Wp2=np.ascontiguousarray(Wp2),
                  Wa2=np.ascontiguousarray(Wa2), bias=bias)
    common["Wa1p"] = np.ascontiguousarray(Wa1)
    NLOC = cfg.NLOC
    xsc = np.maximum(np.abs(x).max(axis=1, keepdims=True), 1e-30) / 127.0
    xq = np.round(x / xsc).astype(np.int8)
    in_maps = [
        dict(common,
             x=np.ascontiguousarray(xq[c * NLOC:(c + 1) * NLOC]),
             xscale=np.ascontiguousarray(xsc[c * NLOC:(c + 1) * NLOC]),
             pos=np.ascontiguousarray(pos[c * NLOC:(c + 1) * NLOC]),
             **p)
        for c, p in enumerate(packs)
    ]
    return in_maps, nchunk


def kernel(**inputs):
    cfg = CFG
    in_maps, nchunk = _build_inputs(inputs, cfg)
    nc = _build(cfg, nchunk)
    res = run_bass_kernel_spmd(nc, in_maps, list(range(cfg.M)))
    y = np.concatenate(
        [res.results[c]["y"][: cfg.NLOC].astype(np.float32) for c in range(cfg.M)],
        axis=0,
    )
    return y


# revision 45
# speedup vs baseline: 1.2382x; 1.2382x over previous
"""Trainium2 Bass kernel for nn_ClusterEncoder (PointTransformerConv-style
GNN message passing), 8-core SPMD.

The axon-tunneled host<->device wire (~45 MB/s up, ~29 MB/s down) dominates
wall time, so the design minimizes shipped bytes end to end:
  * x ships int8 with per-row scales (6.4 MB total instead of 8x25.6 MB
    replicated f32); pos ships sharded f32; per-edge payload is just
    srcid (u16) + dstloc (u8) + per-chunk aux (u16) -- posd and dst ids are
    derived on device; y returns fp16.  A persistent XLA compilation cache
    makes repeat invocations skip the HLO->NEFF pipeline.

Strategy (edges sharded by destination node):
  * Host: sort edges by dst, split nodes into 8 equal contiguous ranges
    (edge counts balance to ~0.3% for this random graph). Within a core,
    pack CONTIGUOUS node windows [n0, n0+cnt) into "chunks" of <=128 nodes
    and <=CHUNK_E edges; pad each chunk's edge list to CHUNK_E slots.
  * Device, phase 1 (x sharded by node range): dequantize x, then
    U = x_loc @ (W_dst@Wa1) -> local DRAM [NLOC, 64+2] with pos appended
    (dst rows are core-local by construction), VH_loc = x_loc @
    [W_src@Wa1 | W_lin] with pos appended -> AllGather over NeuronLink
    into full VH [N, 194] for src gathers.
  * Device, phase 2 (per chunk of 16 x 128-edge tiles):
      - dst = min(dstloc + n0, NLOC-1) (pad lanes clamp to a valid row and
        contribute zero through the zero indicator row),
      - gather VH rows by global src and U rows by local dst,
      - posd = pos[dst] - pos[src] from the gathered columns, transposed
        into a [2, CHUNK_E] tile for the pos MLP,
      - gd = U[dst] - V[src]  (attn-layer-1 folded through node features),
      - pos MLP: t_p1 = relu(Wp1^T posd^T + bp1), delta = relu(Wp2^T t_p1 + bp2),
      - z1 = Wa1^T delta;  t_a = relu(z1 + gd^T + ba1),
      - logits = relu(Wa2^T t_a + ba2);  e = exp(logits - SHIFT)
        (softmax max-subtraction replaced by a constant shift -- exactly
        equivalent math since the shift cancels in e/sum(e); logits are
        relu-bounded so no overflow),
      - one-hot indicator per tile from local dst index (iota + is_equal),
      - segment-sum via matmul: acc[n, 0:128] += ind^T @ (e*(H[src]+delta))^T,
        acc[n, 128:256] += ind^T @ e^T   (numerator and normalizer together),
      - out = relu(NUM / (s + eps)); indirect-scatter fp16 rows to y.
  * Softmax segments are core-local by construction; the only collective is
    the phase-1 AllGather.
"""
import sys
from dataclasses import dataclass
from math import ceil

if "/opt/trn_rl_repo" not in sys.path:
    sys.path.insert(0, "/opt/trn_rl_repo")

import ml_dtypes
import numpy as np
import jax

# Persistent XLA compilation cache: repeated kernel invocations (fresh
# jax.jit closures inside run_bass_kernel_spmd) hit the disk cache instead
# of re-running the HLO pipeline + BIR->NEFF hook (~0.8s/call -> ~0.04s).
jax.config.update("jax_compilation_cache_dir", "/tmp/jax_comp_cache")
jax.config.update("jax_persistent_cache_min_compile_time_secs", 0)
try:
    jax.config.update("jax_persistent_cache_min_entry_size_bytes", 0)
except Exception:
    pass

import concourse.bass as bass
import concourse.mybir as mybir
import concourse.tile as tile
from concourse import bacc
from concourse.bass import IndirectOffsetOnAxis
from concourse.bass_utils import run_bass_kernel_spmd
from concourse.masks import make_identity

f32 = mybir.dt.float32
f32r = mybir.dt.float32r
bf16 = mybir.dt.bfloat16
f16 = mybir.dt.float16
i32 = mybir.dt.int32
i8 = mybir.dt.int8
u16 = mybir.dt.uint16
u8 = mybir.dt.uint8
AF = mybir.ActivationFunctionType
ALU = mybir.AluOpType


@dataclass
class Cfg:
    N: int = 50000
    C: int = 128
    PH: int = 64
    AH: int = 64
    DIM: int = 2
    M: int = 8            # cores
    T: int = 16           # 128-edge tiles per chunk
    TB: int = 4           # tiles per matmul block (block = 512 edges)
    SHIFT: float = 8.0
    EPS: float = 1e-12
    mm_dt: object = f32r  # matmul compute dtype (f32r: 1 cyc/row at free>=256)

    @property
    def NLOC(self):
        return self.N // self.M

    @property
    def CHUNK_E(self):
        return self.T * 128

    @property
    def OUT_ROWS(self):
        return self.NLOC + 1  # +1 trash row for padded scatter lanes


CFG = Cfg()


# ---------------------------------------------------------------- host pack
def _pack(x, pos, edge_index, cfg):
    """Sort/shard/chunk edges; returns per-core input dicts (minus weights).

    Chunks are CONTIGUOUS local-node windows [n0, n0+cnt) with cnt<=128 and
    total edge count <=CHUNK_E (isolated nodes just occupy a row and output
    zero).  Contiguity lets the device derive the gather ids:
    dst = min(dstloc + n0, NLOC-1), so only dstloc (u8) + aux (u16 outrow/n0)
    ship per edge tile.  posd is computed on device from pos columns carried
    in the gathered U/VH rows, so no per-edge pos payload ships at all.
    """
    src = np.asarray(edge_index[0], np.int64)
    dst = np.asarray(edge_index[1], np.int64)
    order = np.argsort(dst, kind="stable")
    s_s = src[order]
    d_s = dst[order]

    NLOC = cfg.NLOC
    bounds = np.searchsorted(d_s, np.arange(cfg.M + 1) * NLOC)

    cores = []
    for c in range(cfg.M):
        lo, hi = bounds[c], bounds[c + 1]
        dloc = d_s[lo:hi] - c * NLOC
        deg = np.bincount(dloc, minlength=NLOC)
        chunks = []  # (n0, cnt, e0, e1) ; e relative to lo
        n0, e0 = 0, 0
        while n0 < NLOC:
            cnt, cur_e = 0, 0
            while (n0 + cnt < NLOC and cnt < 128
                   and cur_e + deg[n0 + cnt] <= cfg.CHUNK_E):
                cur_e += int(deg[n0 + cnt])
                cnt += 1
            assert cnt > 0, f"degree {deg[n0]} exceeds chunk capacity"
            chunks.append((n0, cnt, e0, e0 + cur_e))
            n0 += cnt
            e0 += cur_e
        cores.append((lo, chunks, dloc))

    NCHUNK = max(len(ch) for _, ch, _ in cores)

    in_maps = []
    for c in range(cfg.M):
        lo, chunks, dloc = cores[c]
        srcid = np.zeros((NCHUNK, 128, cfg.T), np.uint16)
        dstloc = np.full((NCHUNK, 128, cfg.T), 255, np.uint8)  # 255 = pad lane
        aux = np.zeros((NCHUNK, 128, 2), np.uint16)
        aux[:, :, 0] = cfg.NLOC  # default scatter row = trash
        for k, (n0, cnt, e0, e1) in enumerate(chunks):
            ecnt = e1 - e0
            g0, g1 = lo + e0, lo + e1
            j = np.arange(ecnt)
            t_idx = j >> 7
            lane = j & 127
            srcid[k, lane, t_idx] = s_s[g0:g1].astype(np.uint16)
            dstloc[k, lane, t_idx] = (dloc[e0:e1] - n0).astype(np.uint8)
            aux[k, :cnt, 0] = (n0 + np.arange(cnt)).astype(np.uint16)
            aux[k, :, 1] = n0
        in_maps.append(dict(srcid=srcid, dstloc=dstloc, aux=aux))
    return in_maps, NCHUNK


# ---------------------------------------------------------------- program
def _build(cfg, nchunk):
    nc = bacc.Bacc(None, target_bir_lowering=False)
    N, C, PH, AH, DIM = cfg.N, cfg.C, cfg.PH, cfg.AH, cfg.DIM
    NLOC = cfg.NLOC
    mdt = cfg.mm_dt

    x_d = nc.declare_dram_parameter("x", [NLOC, C], i8, isOutput=False)
    xs_d = nc.declare_dram_parameter("xscale", [NLOC, 1], f32, isOutput=False)
    pos_d = nc.declare_dram_parameter("pos", [NLOC, DIM], f32, isOutput=False)
    wnode_d = nc.declare_dram_parameter("Wnode", [C, 2 * AH + C], f32, isOutput=False)
    wp1_d = nc.declare_dram_parameter("Wp1", [DIM, PH], f32, isOutput=False)
    wp2_d = nc.declare_dram_parameter("Wp2", [PH, C], f32, isOutput=False)
    wa1_d = nc.declare_dram_parameter("Wa1p", [C, AH], f32, isOutput=False)
    wa2_d = nc.declare_dram_parameter("Wa2", [AH, C], f32, isOutput=False)
    bias_d = nc.declare_dram_parameter("bias", [128, 5], f32, isOutput=False)
    src_d = nc.declare_dram_parameter("srcid", [nchunk, 128, cfg.T], u16, isOutput=False)
    dl_d = nc.declare_dram_parameter("dstloc", [nchunk, 128, cfg.T], u8, isOutput=False)
    aux_d = nc.declare_dram_parameter("aux", [nchunk, 128, 2], u16, isOutput=False)
    y_d = nc.declare_dram_parameter("y", [cfg.OUT_ROWS, C], f16, isOutput=True)

    # U rows: [U(AH) | pos(DIM)] ; VH rows: [V(AH) | H(C) | pos(DIM)]
    UW = AH + DIM
    VW = AH + C + DIM
    U_d = nc.dram_tensor("U", [NLOC, UW], f32)         # x_loc @ (W_dst@Wa1)
    VHloc_d = nc.dram_tensor("VHl", [NLOC, VW], f32)
    VH_d = nc.dram_tensor("VH", [N, VW], f32, addr_space="Shared")

    NB = cfg.T // cfg.TB  # blocks per chunk
    BLK = cfg.TB * 128

    with tile.TileContext(nc) as tc:
        with tc.tile_pool(name="const", bufs=1) as cp:
            wnode_s = cp.tile([C, 2 * AH + C], f32)
            nc.sync.dma_start(out=wnode_s[:], in_=wnode_d[:, :])
            wp1_s = cp.tile([DIM, PH], f32)
            nc.sync.dma_start(out=wp1_s[:], in_=wp1_d[:, :])
            wp2_s = cp.tile([PH, C], f32)
            nc.sync.dma_start(out=wp2_s[:], in_=wp2_d[:, :])
            wa2_s = cp.tile([AH, C], f32)
            nc.sync.dma_start(out=wa2_s[:], in_=wa2_d[:, :])
            bias_s = cp.tile([128, 5], f32)
            nc.sync.dma_start(out=bias_s[:], in_=bias_d[:, :])
            ident_s = cp.tile([128, 128], f32)
            make_identity(nc, ident_s[:])
            iota_i = cp.tile([128, 128], i32)
            nc.gpsimd.iota(iota_i[:], pattern=[[1, 128]], base=0, channel_multiplier=0)
            iota_s = cp.tile([128, 128], f32)
            nc.vector.tensor_copy(iota_s[:], iota_i[:])
            wa1_s = cp.tile([C, AH], f32)
            nc.sync.dma_start(out=wa1_s[:], in_=wa1_d[:, :])

            # fp32r matmul operands must be produced rounded-to-f32r: make
            # rounded copies of the stationary weights once.
            if mdt is f32r:
                wnode_m = cp.tile([C, 2 * AH + C], f32r)
                nc.vector.tensor_copy(wnode_m[:], wnode_s[:])
                wp1_m = cp.tile([DIM, PH], f32r)
                nc.vector.tensor_copy(wp1_m[:], wp1_s[:])
                wp2_m = cp.tile([PH, C], f32r)
                nc.vector.tensor_copy(wp2_m[:], wp2_s[:])
                wa1_m = cp.tile([C, AH], f32r)
                nc.vector.tensor_copy(wa1_m[:], wa1_s[:])
                wa2_m = cp.tile([AH, C], f32r)
                nc.vector.tensor_copy(wa2_m[:], wa2_s[:])
            else:
                wnode_m, wp1_m, wp2_m, wa1_m, wa2_m = wnode_s, wp1_s, wp2_s, wa1_s, wa2_s

            # ------- phase 1: local node features U / VH_loc, then AllGather
            with tc.tile_pool(name="p1", bufs=3) as p1, \
                 tc.tile_pool(name="p1ps", bufs=2, space="PSUM") as p1ps:
                nt = ceil(NLOC / 128)
                for t in range(nt):
                    r0 = t * 128
                    rows = min(128, NLOC - r0)
                    xq_t = p1.tile([128, C], i8, tag="xq")
                    nc.sync.dma_start(out=xq_t[:rows], in_=x_d[r0:r0 + rows, :])
                    xsc_t = p1.tile([128, 1], f32, tag="xsc")
                    nc.sync.dma_start(out=xsc_t[:rows], in_=xs_d[r0:r0 + rows, :])
                    xt = p1.tile([128, C], f32, tag="xt")
                    nc.vector.tensor_scalar(xt[:rows], xq_t[:rows],
                                            xsc_t[:rows, 0:1], None, op0=ALU.mult)
                    xT_p = p1ps.tile([128, 128], f32, tag="xT")
                    nc.tensor.transpose(xT_p[:, :rows], xt[:rows, :], ident_s[:rows, :rows])
                    xT_s = p1.tile([128, 128], mdt, tag="xTs")
                    nc.vector.tensor_copy(xT_s[:, :rows], xT_p[:, :rows])
                    uvh_p = p1ps.tile([128, 2 * AH + C], f32, tag="uvh")
                    nc.tensor.matmul(uvh_p[:rows, :], lhsT=xT_s[:, :rows],
                                     rhs=wnode_m[:], start=True, stop=True)
                    uvh_s = p1.tile([128, 2 * AH + C], f32, tag="uvhs")
                    nc.scalar.activation(uvh_s[:rows, :], uvh_p[:rows, :], AF.Copy)
                    post = p1.tile([128, DIM], f32, tag="post")
                    nc.sync.dma_start(out=post[:rows], in_=pos_d[r0:r0 + rows, :])
                    nc.sync.dma_start(out=U_d[r0:r0 + rows, 0:AH], in_=uvh_s[:rows, 0:AH])
                    nc.sync.dma_start(out=U_d[r0:r0 + rows, AH:UW], in_=post[:rows])
                    nc.sync.dma_start(out=VHloc_d[r0:r0 + rows, 0:AH + C], in_=uvh_s[:rows, AH:])
                    nc.sync.dma_start(out=VHloc_d[r0:r0 + rows, AH + C:VW], in_=post[:rows])

            nc.gpsimd.collective_compute(
                "AllGather",
                mybir.AluOpType.bypass,
                replica_groups=[list(range(cfg.M))],
                ins=[VHloc_d[:, :]],
                outs=[VH_d[:, :]],
            )

            # ---------------- phase 2: edges ----------------
            with tc.tile_pool(name="eb", bufs=3) as eb, \
                 tc.tile_pool(name="ebg", bufs=3) as ebg, \
                 tc.tile_pool(name="ps_acc", bufs=2, space="PSUM") as ps_acc, \
                 tc.tile_pool(name="ps_b", bufs=1, space="PSUM") as ps_b, \
                 tc.tile_pool(name="ps_c", bufs=1, space="PSUM") as ps_c, \
                 tc.tile_pool(name="ps_m", bufs=1, space="PSUM") as ps_m, \
                 tc.tile_pool(name="ps_n", bufs=1, space="PSUM") as ps_n, \
                 tc.tile_pool(name="ps_t", bufs=2, space="PSUM") as ps_t:
                with tc.For_i(0, nchunk) as k:
                    src16_s = eb.tile([128, cfg.T], u16, tag="src16")
                    nc.sync.dma_start(out=src16_s[:], in_=src_d[k, :, :])
                    src_s = eb.tile([128, cfg.T], i32, tag="src")
                    nc.vector.tensor_copy(src_s[:], src16_s[:])
                    dl8_s = eb.tile([128, cfg.T], u8, tag="dl8")
                    nc.sync.dma_start(out=dl8_s[:], in_=dl_d[k, :, :])
                    dl_s = eb.tile([128, cfg.T], f32, tag="dl")
                    nc.vector.tensor_copy(dl_s[:], dl8_s[:])
                    aux16_s = eb.tile([128, 2], u16, tag="aux16")
                    nc.sync.dma_start(out=aux16_s[:], in_=aux_d[k, :, :])
                    aux_s = eb.tile([128, 2], i32, tag="aux")
                    nc.vector.tensor_copy(aux_s[:], aux16_s[:])
                    aux_f = eb.tile([128, 2], f32, tag="auxf")
                    nc.vector.tensor_copy(aux_f[:], aux16_s[:])
                    # dst = min(dstloc + n0, NLOC-1): pad lanes (255) clamp to a
                    # valid row, contributing 0 through the zero indicator row.
                    dstf_s = eb.tile([128, cfg.T], f32, tag="dstf")
                    nc.vector.tensor_scalar(dstf_s[:], dl_s[:], aux_f[:, 1:2],
                                            float(NLOC - 1),
                                            op0=ALU.add, op1=ALU.min)
                    dst_s = eb.tile([128, cfg.T], i32, tag="dst")
                    nc.vector.tensor_copy(dst_s[:], dstf_s[:])
                    or_s = aux_s  # column 0 = scatter rows
                    pd_s = eb.tile([DIM, cfg.CHUNK_E], mdt, tag="pd")

                    acc_p = ps_acc.tile([128, 2 * C], f32, tag="acc")

                    for b in range(NB):
                        esl = slice(b * BLK, (b + 1) * BLK)
                        # gathers for this block, one [128,1]-offset DMA per tile
                        vhg_b = ebg.tile([128, cfg.TB, VW], f32, tag="vhgb")
                        ug_b = ebg.tile([128, cfg.TB, UW], f32, tag="ugb")
                        for tt in range(cfg.TB):
                            ti = b * cfg.TB + tt
                            nc.gpsimd.indirect_dma_start(
                                out=vhg_b[:, tt, :], out_offset=None, in_=VH_d[:],
                                in_offset=IndirectOffsetOnAxis(
                                    ap=src_s[:, ti:ti + 1], axis=0))
                            nc.gpsimd.indirect_dma_start(
                                out=ug_b[:, tt, :], out_offset=None, in_=U_d[:],
                                in_offset=IndirectOffsetOnAxis(
                                    ap=dst_s[:, ti:ti + 1], axis=0))
                        vhgs = [vhg_b[:, tt, :] for tt in range(cfg.TB)]
                        ugs = [ug_b[:, tt, :] for tt in range(cfg.TB)]

                        # posd = pos[dst] - pos[src] from gathered columns;
                        # transpose [128,2] -> [2,128] into the chunk pd tile
                        posd_s = eb.tile([128, cfg.TB, DIM], f32, tag="posd")
                        nc.vector.tensor_tensor(
                            posd_s[:, :, :], ug_b[:, :, AH:UW],
                            vhg_b[:, :, AH + C:VW], op=ALU.subtract)
                        for tt in range(cfg.TB):
                            ti = b * cfg.TB + tt
                            pdT_p = ps_t.tile([128, 128], f32, tag="tr")
                            nc.tensor.transpose(pdT_p[:DIM, :], posd_s[:, tt, :],
                                                ident_s[:])
                            nc.scalar.activation(
                                pd_s[:, ti * 128:(ti + 1) * 128],
                                pdT_p[:DIM, :], AF.Copy)

                        # pos MLP
                        tp1_p = ps_m.tile([PH, BLK], f32, tag="tp1")
                        nc.tensor.matmul(tp1_p[:], lhsT=wp1_m[:],
                                         rhs=pd_s[:, esl], start=True, stop=True)
                        tp1_s = eb.tile([PH, BLK], mdt, tag="tp1s")
                        nc.scalar.activation(tp1_s[:], tp1_p[:], AF.Relu, bias=bias_s[0:PH, 0:1])
                        del_p = ps_b.tile([C, BLK], f32, tag="delp")
                        nc.tensor.matmul(del_p[:], lhsT=wp2_m[:],
                                         rhs=tp1_s[:], start=True, stop=True)
                        del_s = eb.tile([C, BLK], f32, tag="dels")
                        nc.scalar.activation(del_s[:], del_p[:], AF.Relu, bias=bias_s[:, 1:2])
                        if mdt is f32r:
                            del_m = eb.tile([C, BLK], f32r, tag="delm")
                            nc.scalar.activation(del_m[:], del_p[:], AF.Relu, bias=bias_s[:, 1:2])
                        else:
                            del_m = del_s

                        # attn layer 1: z1 = Wa1^T delta ; t_a = relu(z1 + gd^T + ba1)
                        z1_p = ps_n.tile([AH, BLK], f32, tag="z1")
                        nc.tensor.matmul(z1_p[:], lhsT=wa1_m[:],
                                         rhs=del_m[:], start=True, stop=True)
                        gd_b = eb.tile([128, cfg.TB, AH], f32, tag="gd")
                        nc.vector.tensor_tensor(gd_b[:, :, :], ug_b[:, :, 0:AH],
                                                vhg_b[:, :, 0:AH],
                                                op=ALU.subtract)
                        gdT_s = eb.tile([AH, BLK], f32, tag="gdT")
                        for tt in range(cfg.TB):
                            gdT_p = ps_t.tile([128, 128], f32, tag="tr")
                            nc.tensor.transpose(gdT_p[:AH, :], gd_b[:, tt, :],
                                                ident_s[:])
                            csl = slice(tt * 128, (tt + 1) * 128)
                            nc.scalar.activation(gdT_s[:, csl], gdT_p[:AH, :], AF.Copy)
                        tsum_s = eb.tile([AH, BLK], f32, tag="tsum")
                        nc.vector.tensor_tensor(tsum_s[:], z1_p[:], gdT_s[:],
                                                op=ALU.add)
                        ta_s = eb.tile([AH, BLK], mdt, tag="ta")
                        nc.scalar.activation(ta_s[:], tsum_s[:], AF.Relu, bias=bias_s[0:AH, 2:3])

                        # attn layer 2 + exp
                        al_p = ps_c.tile([C, BLK], f32, tag="al")
                        nc.tensor.matmul(al_p[:], lhsT=wa2_m[:],
                                         rhs=ta_s[:], start=True, stop=True)
                        ar_s = eb.tile([C, BLK], f32, tag="ar")
                        nc.scalar.activation(ar_s[:], al_p[:], AF.Relu, bias=bias_s[:, 3:4])
                        e_s = eb.tile([C, BLK], f32, tag="e")
                        nc.scalar.activation(e_s[:], ar_s[:], AF.Exp, bias=bias_s[:, 4:5])
                        ew2_s = eb.tile([C, BLK], f32, tag="ew2")
                        nc.vector.tensor_tensor(ew2_s[:], e_s[:], del_s[:], op=ALU.mult)
                        del del_s  # f32 copy only feeds ew2

                        # per-tile: transpose, assemble [ew | e]^T, indicator, seg-matmul
                        for tt in range(cfg.TB):
                            ti = b * cfg.TB + tt
                            csl = slice(tt * 128, (tt + 1) * 128)
                            eT_p = ps_t.tile([128, 128], f32, tag="tr")
                            nc.tensor.transpose(eT_p[:], e_s[:, csl], ident_s[:])
                            ew2T_p = ps_t.tile([128, 128], f32, tag="tr")
                            nc.tensor.transpose(ew2T_p[:], ew2_s[:, csl], ident_s[:])
                            ewe_s = eb.tile([128, 2 * C], mdt, tag="ewe")
                            nc.vector.tensor_copy(ewe_s[:, C:], eT_p[:])
                            tmp_s = eb.tile([128, C], f32, tag="tmp")
                            nc.vector.tensor_tensor(tmp_s[:], eT_p[:],
                                                    vhgs[tt][:, AH:AH + C],
                                                    op=ALU.mult)
                            nc.vector.tensor_tensor(ewe_s[:, 0:C], tmp_s[:], ew2T_p[:],
                                                    op=ALU.add)
                            ind_s = eb.tile([128, 128], mdt, tag="ind")
                            nc.vector.tensor_scalar(ind_s[:], iota_s[:], dl_s[:, ti:ti + 1],
                                                    None, op0=ALU.is_equal)
                            nc.tensor.matmul(acc_p[:], lhsT=ind_s[:],
                                             rhs=ewe_s[:],
                                             start=(ti == 0), stop=(ti == cfg.T - 1))

                    # finalize chunk
                    sp_s = eb.tile([128, C], f32, tag="sp")
                    nc.vector.tensor_scalar_add(sp_s[:], acc_p[:, C:], cfg.EPS)
                    rp_s = eb.tile([128, C], f32, tag="rp")
                    nc.vector.reciprocal(rp_s[:], sp_s[:])
                    o_s = eb.tile([128, C], f32, tag="o")
                    nc.vector.tensor_tensor(o_s[:], acc_p[:, 0:C], rp_s[:], op=ALU.mult)
                    o2_s = eb.tile([128, C], f16, tag="o2")
                    nc.scalar.activation(o2_s[:], o_s[:], AF.Relu)
                    nc.gpsimd.indirect_dma_start(
                        out=y_d[:], out_offset=IndirectOffsetOnAxis(ap=or_s[:, :1], axis=0),
                        in_=o2_s[:], in_offset=None)
    nc.finalize()
    return nc


def _build_inputs(inputs, cfg):
    x = np.ascontiguousarray(np.asarray(inputs["x"], np.float32))
    pos = np.ascontiguousarray(np.asarray(inputs["pos"], np.float32))
    W_lin = np.asarray(inputs["W_lin"], np.float32)
    W_src = np.asarray(inputs["W_src"], np.float32)
    W_dst = np.asarray(inputs["W_dst"], np.float32)
    Wp1 = np.asarray(inputs["Wp1"], np.float32)
    bp1 = np.asarray(inputs["bp1"], np.float32)
    Wp2 = np.asarray(inputs["Wp2"], np.float32)
    bp2 = np.asarray(inputs["bp2"], np.float32)
    Wa1 = np.asarray(inputs["Wa1"], np.float32)
    ba1 = np.asarray(inputs["ba1"], np.float32)
    Wa2 = np.asarray(inputs["Wa2"], np.float32)
    ba2 = np.asarray(inputs["ba2"], np.float32)

    Wda = (W_dst @ Wa1).astype(np.float32)   # [C, AH]
    Wsa = (W_src @ Wa1).astype(np.float32)
    wnode = np.concatenate([Wda, Wsa, W_lin], axis=1)  # [C, 2AH + C]
    bias = np.zeros((128, 5), np.float32)
    bias[: cfg.PH, 0] = bp1
    bias[: cfg.C, 1] = bp2
    bias[: cfg.AH, 2] = ba1
    bias[: cfg.C, 3] = ba2
    bias[:, 4] = -cfg.SHIFT

    packs, nchunk = _pack(x, pos, inputs["edge_index"], cfg)
    common = dict(Wnode=np.ascontiguousarray(wnode),
                  Wp1=np.ascontiguousarray(Wp1), Wp2=np.ascontiguousarray(Wp2),
                  Wa2=np.ascontiguousarray(Wa2), bias=bias)
    common["Wa1p"] = np.ascontiguousarray(Wa1)
    NLOC = cfg.NLOC
    xsc = np.maximum(np.abs(x).max(axis=1, keepdims=True), 1e-30) / 127.0
    xq = np.round(x / xsc).astype(np.int8)
    in_maps = [
        dict(common,
             x=np.ascontiguousarray(xq[c * NLOC:(c + 1) * NLOC]),
             xscale=np.ascontiguousarray(xsc[c * NLOC:(c + 1) * NLOC]),
             pos=np.ascontiguousarray(pos[c * NLOC:(c + 1) * NLOC]),
             **p)
        for c, p in enumerate(packs)
    ]
    return in_maps, nchunk


def kernel(**inputs):
    cfg = CFG
    in_maps, nchunk = _build_inputs(inputs, cfg)
    nc = _build(cfg, nchunk)
    res = run_bass_kernel_spmd(nc, in_maps, list(range(cfg.M)))
    y = np.concatenate(
        [res.results[c]["y"][: cfg.NLOC].astype(np.float32) for c in range(cfg.M)],
        axis=0,
    )
    return y


# revision 46
# speedup vs baseline: 1.2688x; 1.0247x over previous
"""Trainium2 Bass kernel for nn_ClusterEncoder (PointTransformerConv-style
GNN message passing), 8-core SPMD.

The axon-tunneled host<->device wire (~45 MB/s up, ~29 MB/s down) dominates
wall time, so the design minimizes shipped bytes end to end:
  * x ships int8 with per-row scales (6.4 MB total instead of 8x25.6 MB
    replicated f32); pos ships sharded f32; per-edge payload is just
    srcid (u16) + dstloc (u8) + per-chunk aux (u16) -- posd and dst ids are
    derived on device; y returns fp16.  A persistent XLA compilation cache
    makes repeat invocations skip the HLO->NEFF pipeline.

Strategy (edges sharded by destination node):
  * Host: sort edges by dst, split nodes into 8 equal contiguous ranges
    (edge counts balance to ~0.3% for this random graph). Within a core,
    pack CONTIGUOUS node windows [n0, n0+cnt) into "chunks" of <=128 nodes
    and <=CHUNK_E edges; pad each chunk's edge list to CHUNK_E slots.
  * Device, phase 1 (x sharded by node range): dequantize x, then
    U = x_loc @ (W_dst@Wa1) -> local DRAM [NLOC, 64+2] with pos appended
    (dst rows are core-local by construction), VH_loc = x_loc @
    [W_src@Wa1 | W_lin] with pos appended -> AllGather over NeuronLink
    into full VH [N, 194] for src gathers.
  * Device, phase 2 (per chunk of 16 x 128-edge tiles):
      - dst = min(dstloc + n0, NLOC-1) (pad lanes clamp to a valid row and
        contribute zero through the zero indicator row),
      - gather VH rows by global src and U rows by local dst,
      - posd = pos[dst] - pos[src] from the gathered columns, transposed
        into a [2, CHUNK_E] tile for the pos MLP,
      - gd = U[dst] - V[src]  (attn-layer-1 folded through node features),
      - pos MLP: t_p1 = relu(Wp1^T posd^T + bp1), delta = relu(Wp2^T t_p1 + bp2),
      - z1 = Wa1^T delta;  t_a = relu(z1 + gd^T + ba1),
      - logits = relu(Wa2^T t_a + ba2);  e = exp(logits - SHIFT)
        (softmax max-subtraction replaced by a constant shift -- exactly
        equivalent math since the shift cancels in e/sum(e); logits are
        relu-bounded so no overflow),
      - one-hot indicator per tile from local dst index (iota + is_equal),
      - segment-sum via matmul: acc[n, 0:128] += ind^T @ (e*(H[src]+delta))^T,
        acc[n, 128:256] += ind^T @ e^T   (numerator and normalizer together),
      - out = relu(NUM / (s + eps)); indirect-scatter fp16 rows to y.
  * Softmax segments are core-local by construction; the only collective is
    the phase-1 AllGather.
"""
import sys
from dataclasses import dataclass
from math import ceil

if "/opt/trn_rl_repo" not in sys.path:
    sys.path.insert(0, "/opt/trn_rl_repo")

import ml_dtypes
import numpy as np
import jax

# Persistent XLA compilation cache: repeated kernel invocations (fresh
# jax.jit closures inside run_bass_kernel_spmd) hit the disk cache instead
# of re-running the HLO pipeline + BIR->NEFF hook (~0.8s/call -> ~0.04s).
jax.config.update("jax_compilation_cache_dir", "/tmp/jax_comp_cache")
jax.config.update("jax_persistent_cache_min_compile_time_secs", 0)
try:
    jax.config.update("jax_persistent_cache_min_entry_size_bytes", 0)
except Exception:
    pass

import concourse.bass as bass
import concourse.mybir as mybir
import concourse.tile as tile
from concourse import bacc
from concourse.bass import IndirectOffsetOnAxis
from concourse.bass_utils import run_bass_kernel_spmd
from concourse.masks import make_identity

f32 = mybir.dt.float32
f32r = mybir.dt.float32r
bf16 = mybir.dt.bfloat16
f16 = mybir.dt.float16
i32 = mybir.dt.int32
i8 = mybir.dt.int8
u16 = mybir.dt.uint16
u8 = mybir.dt.uint8
AF = mybir.ActivationFunctionType
ALU = mybir.AluOpType


@dataclass
class Cfg:
    N: int = 50000
    C: int = 128
    PH: int = 64
    AH: int = 64
    DIM: int = 2
    M: int = 8            # cores
    T: int = 16           # 128-edge tiles per chunk
    TB: int = 4           # tiles per matmul block (block = 512 edges)
    SHIFT: float = 8.0
    EPS: float = 1e-12
    mm_dt: object = f32r  # matmul compute dtype (f32r: 1 cyc/row at free>=256)

    @property
    def NLOC(self):
        return self.N // self.M

    @property
    def CHUNK_E(self):
        return self.T * 128

    @property
    def OUT_ROWS(self):
        return self.NLOC + 1  # +1 trash row for padded scatter lanes


CFG = Cfg()


# ---------------------------------------------------------------- host pack
def _pack(x, pos, edge_index, cfg):
    """Sort/shard/chunk edges; returns per-core input dicts (minus weights).

    Chunks are CONTIGUOUS local-node windows [n0, n0+cnt) with cnt<=128 and
    total edge count <=CHUNK_E (isolated nodes just occupy a row and output
    zero).  Contiguity lets the device derive the gather ids:
    dst = min(dstloc + n0, NLOC-1), so only dstloc (u8) + aux (u16 outrow/n0)
    ship per edge tile.  posd is computed on device from pos columns carried
    in the gathered U/VH rows, so no per-edge pos payload ships at all.
    """
    src = np.asarray(edge_index[0], np.int64)
    dst = np.asarray(edge_index[1], np.int64)
    order = np.argsort(dst, kind="stable")
    s_s = src[order]
    d_s = dst[order]

    NLOC = cfg.NLOC
    bounds = np.searchsorted(d_s, np.arange(cfg.M + 1) * NLOC)

    cores = []
    for c in range(cfg.M):
        lo, hi = bounds[c], bounds[c + 1]
        dloc = d_s[lo:hi] - c * NLOC
        deg = np.bincount(dloc, minlength=NLOC)
        chunks = []  # (n0, cnt, e0, e1) ; e relative to lo
        n0, e0 = 0, 0
        while n0 < NLOC:
            cnt, cur_e = 0, 0
            while (n0 + cnt < NLOC and cnt < 128
                   and cur_e + deg[n0 + cnt] <= cfg.CHUNK_E):
                cur_e += int(deg[n0 + cnt])
                cnt += 1
            assert cnt > 0, f"degree {deg[n0]} exceeds chunk capacity"
            chunks.append((n0, cnt, e0, e0 + cur_e))
            n0 += cnt
            e0 += cur_e
        cores.append((lo, chunks, dloc))

    NCHUNK = max(len(ch) for _, ch, _ in cores)

    in_maps = []
    for c in range(cfg.M):
        lo, chunks, dloc = cores[c]
        srcid = np.zeros((NCHUNK, 128, cfg.T), np.uint16)
        dstloc = np.full((NCHUNK, 128, cfg.T), 255, np.uint8)  # 255 = pad lane
        aux = np.zeros((NCHUNK, 128, 2), np.uint16)
        aux[:, :, 0] = cfg.NLOC  # default scatter row = trash
        for k, (n0, cnt, e0, e1) in enumerate(chunks):
            ecnt = e1 - e0
            g0, g1 = lo + e0, lo + e1
            j = np.arange(ecnt)
            t_idx = j >> 7
            lane = j & 127
            srcid[k, lane, t_idx] = s_s[g0:g1].astype(np.uint16)
            dstloc[k, lane, t_idx] = (dloc[e0:e1] - n0).astype(np.uint8)
            aux[k, :cnt, 0] = (n0 + np.arange(cnt)).astype(np.uint16)
            aux[k, :, 1] = n0
        in_maps.append(dict(srcid=srcid, dstloc=dstloc, aux=aux))
    return in_maps, NCHUNK


# ---------------------------------------------------------------- program
def _build(cfg, nchunk):
    nc = bacc.Bacc(None, target_bir_lowering=False)
    N, C, PH, AH, DIM = cfg.N, cfg.C, cfg.PH, cfg.AH, cfg.DIM
    NLOC = cfg.NLOC
    mdt = cfg.mm_dt

    x_d = nc.declare_dram_parameter("x", [NLOC, C], i8, isOutput=False)
    xs_d = nc.declare_dram_parameter("xscale", [NLOC, 1], f32, isOutput=False)
    pos_d = nc.declare_dram_parameter("pos", [NLOC, DIM], f32, isOutput=False)
    wnode_d = nc.declare_dram_parameter("Wnode", [C, 2 * AH + C], f32, isOutput=False)
    wp1_d = nc.declare_dram_parameter("Wp1", [DIM, PH], f32, isOutput=False)
    wp2_d = nc.declare_dram_parameter("Wp2", [PH, C], f32, isOutput=False)
    wa1_d = nc.declare_dram_parameter("Wa1p", [C, AH], f32, isOutput=False)
    wa2_d = nc.declare_dram_parameter("Wa2", [AH, C], f32, isOutput=False)
    bias_d = nc.declare_dram_parameter("bias", [128, 5], f32, isOutput=False)
    src_d = nc.declare_dram_parameter("srcid", [nchunk, 128, cfg.T], u16, isOutput=False)
    dl_d = nc.declare_dram_parameter("dstloc", [nchunk, 128, cfg.T], u8, isOutput=False)
    aux_d = nc.declare_dram_parameter("aux", [nchunk, 128, 2], u16, isOutput=False)
    y_d = nc.declare_dram_parameter("y", [cfg.OUT_ROWS, C], f16, isOutput=True)

    # U rows: [U(AH) | pos(DIM)] ; VH rows: [V(AH) | H(C) | pos(DIM)]
    UW = AH + DIM
    VW = AH + C + DIM
    U_d = nc.dram_tensor("U", [NLOC, UW], f32)         # x_loc @ (W_dst@Wa1)
    VHloc_d = nc.dram_tensor("VHl", [NLOC, VW], f32)
    VH_d = nc.dram_tensor("VH", [N, VW], f32, addr_space="Shared")

    NB = cfg.T // cfg.TB  # blocks per chunk
    BLK = cfg.TB * 128

    with tile.TileContext(nc) as tc:
        with tc.tile_pool(name="const", bufs=1) as cp:
            wnode_s = cp.tile([C, 2 * AH + C], f32)
            nc.sync.dma_start(out=wnode_s[:], in_=wnode_d[:, :])
            wp1_s = cp.tile([DIM, PH], f32)
            nc.sync.dma_start(out=wp1_s[:], in_=wp1_d[:, :])
            wp2_s = cp.tile([PH, C], f32)
            nc.sync.dma_start(out=wp2_s[:], in_=wp2_d[:, :])
            wa2_s = cp.tile([AH, C], f32)
            nc.sync.dma_start(out=wa2_s[:], in_=wa2_d[:, :])
            bias_s = cp.tile([128, 5], f32)
            nc.sync.dma_start(out=bias_s[:], in_=bias_d[:, :])
            ident_s = cp.tile([128, 128], f32)
            make_identity(nc, ident_s[:])
            iota_i = cp.tile([128, 128], i32)
            nc.gpsimd.iota(iota_i[:], pattern=[[1, 128]], base=0, channel_multiplier=0)
            iota_s = cp.tile([128, 128], f32)
            nc.vector.tensor_copy(iota_s[:], iota_i[:])
            wa1_s = cp.tile([C, AH], f32)
            nc.sync.dma_start(out=wa1_s[:], in_=wa1_d[:, :])

            # fp32r matmul operands must be produced rounded-to-f32r: make
            # rounded copies of the stationary weights once.
            if mdt is f32r:
                wnode_m = cp.tile([C, 2 * AH + C], f32r)
                nc.vector.tensor_copy(wnode_m[:], wnode_s[:])
                wp1_m = cp.tile([DIM, PH], f32r)
                nc.vector.tensor_copy(wp1_m[:], wp1_s[:])
                wp2_m = cp.tile([PH, C], f32r)
                nc.vector.tensor_copy(wp2_m[:], wp2_s[:])
                wa1_m = cp.tile([C, AH], f32r)
                nc.vector.tensor_copy(wa1_m[:], wa1_s[:])
                wa2_m = cp.tile([AH, C], f32r)
                nc.vector.tensor_copy(wa2_m[:], wa2_s[:])
            else:
                wnode_m, wp1_m, wp2_m, wa1_m, wa2_m = wnode_s, wp1_s, wp2_s, wa1_s, wa2_s

            # ------- phase 1: local node features U / VH_loc, then AllGather
            with tc.tile_pool(name="p1", bufs=3) as p1, \
                 tc.tile_pool(name="p1ps", bufs=2, space="PSUM") as p1ps:
                nt = ceil(NLOC / 128)
                for t in range(nt):
                    r0 = t * 128
                    rows = min(128, NLOC - r0)
                    xq_t = p1.tile([128, C], i8, tag="xq")
                    nc.sync.dma_start(out=xq_t[:rows], in_=x_d[r0:r0 + rows, :])
                    xsc_t = p1.tile([128, 1], f32, tag="xsc")
                    nc.sync.dma_start(out=xsc_t[:rows], in_=xs_d[r0:r0 + rows, :])
                    xt = p1.tile([128, C], f32, tag="xt")
                    nc.vector.tensor_scalar(xt[:rows], xq_t[:rows],
                                            xsc_t[:rows, 0:1], None, op0=ALU.mult)
                    xT_p = p1ps.tile([128, 128], f32, tag="xT")
                    nc.tensor.transpose(xT_p[:, :rows], xt[:rows, :], ident_s[:rows, :rows])
                    xT_s = p1.tile([128, 128], mdt, tag="xTs")
                    nc.vector.tensor_copy(xT_s[:, :rows], xT_p[:, :rows])
                    uvh_p = p1ps.tile([128, 2 * AH + C], f32, tag="uvh")
                    nc.tensor.matmul(uvh_p[:rows, :], lhsT=xT_s[:, :rows],
                                     rhs=wnode_m[:], start=True, stop=True)
                    uvh_s = p1.tile([128, 2 * AH + C], f32, tag="uvhs")
                    nc.scalar.activation(uvh_s[:rows, :], uvh_p[:rows, :], AF.Copy)
                    post = p1.tile([128, DIM], f32, tag="post")
                    nc.sync.dma_start(out=post[:rows], in_=pos_d[r0:r0 + rows, :])
                    nc.sync.dma_start(out=U_d[r0:r0 + rows, 0:AH], in_=uvh_s[:rows, 0:AH])
                    nc.sync.dma_start(out=U_d[r0:r0 + rows, AH:UW], in_=post[:rows])
                    nc.sync.dma_start(out=VHloc_d[r0:r0 + rows, 0:AH + C], in_=uvh_s[:rows, AH:])
                    nc.sync.dma_start(out=VHloc_d[r0:r0 + rows, AH + C:VW], in_=post[:rows])

            nc.gpsimd.collective_compute(
                "AllGather",
                mybir.AluOpType.bypass,
                replica_groups=[list(range(cfg.M))],
                ins=[VHloc_d[:, :]],
                outs=[VH_d[:, :]],
            )
            # Hard fence: the phase-2 For_i body lives in its own basic
            # blocks, so the tracker's collective->gather edge does not
            # survive the loop restructure; without this, first-call runs
            # can gather from VH_d before the AllGather lands (NaN).
            tc.strict_bb_all_engine_barrier()

            # ---------------- phase 2: edges ----------------
            with tc.tile_pool(name="eb", bufs=3) as eb, \
                 tc.tile_pool(name="ebg", bufs=3) as ebg, \
                 tc.tile_pool(name="ps_acc", bufs=2, space="PSUM") as ps_acc, \
                 tc.tile_pool(name="ps_b", bufs=1, space="PSUM") as ps_b, \
                 tc.tile_pool(name="ps_c", bufs=1, space="PSUM") as ps_c, \
                 tc.tile_pool(name="ps_m", bufs=1, space="PSUM") as ps_m, \
                 tc.tile_pool(name="ps_n", bufs=1, space="PSUM") as ps_n, \
                 tc.tile_pool(name="ps_t", bufs=2, space="PSUM") as ps_t:
                with tc.For_i(0, nchunk) as k:
                    src16_s = eb.tile([128, cfg.T], u16, tag="src16")
                    nc.sync.dma_start(out=src16_s[:], in_=src_d[k, :, :])
                    src_s = eb.tile([128, cfg.T], i32, tag="src")
                    nc.vector.tensor_copy(src_s[:], src16_s[:])
                    dl8_s = eb.tile([128, cfg.T], u8, tag="dl8")
                    nc.sync.dma_start(out=dl8_s[:], in_=dl_d[k, :, :])
                    dl_s = eb.tile([128, cfg.T], f32, tag="dl")
                    nc.vector.tensor_copy(dl_s[:], dl8_s[:])
                    aux16_s = eb.tile([128, 2], u16, tag="aux16")
                    nc.sync.dma_start(out=aux16_s[:], in_=aux_d[k, :, :])
                    aux_s = eb.tile([128, 2], i32, tag="aux")
                    nc.vector.tensor_copy(aux_s[:], aux16_s[:])
                    aux_f = eb.tile([128, 2], f32, tag="auxf")
                    nc.vector.tensor_copy(aux_f[:], aux16_s[:])
                    # dst = min(dstloc + n0, NLOC-1): pad lanes (255) clamp to a
                    # valid row, contributing 0 through the zero indicator row.
                    dstf_s = eb.tile([128, cfg.T], f32, tag="dstf")
                    nc.vector.tensor_scalar(dstf_s[:], dl_s[:], aux_f[:, 1:2],
                                            float(NLOC - 1),
                                            op0=ALU.add, op1=ALU.min)
                    dst_s = eb.tile([128, cfg.T], i32, tag="dst")
                    nc.vector.tensor_copy(dst_s[:], dstf_s[:])
                    or_s = aux_s  # column 0 = scatter rows
                    pd_s = eb.tile([DIM, cfg.CHUNK_E], mdt, tag="pd")

                    acc_p = ps_acc.tile([128, 2 * C], f32, tag="acc")

                    for b in range(NB):
                        esl = slice(b * BLK, (b + 1) * BLK)
                        # gathers for this block, one [128,1]-offset DMA per tile
                        vhg_b = ebg.tile([128, cfg.TB, VW], f32, tag="vhgb")
                        ug_b = ebg.tile([128, cfg.TB, UW], f32, tag="ugb")
                        for tt in range(cfg.TB):
                            ti = b * cfg.TB + tt
                            nc.gpsimd.indirect_dma_start(
                                out=vhg_b[:, tt, :], out_offset=None, in_=VH_d[:],
                                in_offset=IndirectOffsetOnAxis(
                                    ap=src_s[:, ti:ti + 1], axis=0))
                            nc.gpsimd.indirect_dma_start(
                                out=ug_b[:, tt, :], out_offset=None, in_=U_d[:],
                                in_offset=IndirectOffsetOnAxis(
                                    ap=dst_s[:, ti:ti + 1], axis=0))
                        vhgs = [vhg_b[:, tt, :] for tt in range(cfg.TB)]
                        ugs = [ug_b[:, tt, :] for tt in range(cfg.TB)]

                        # posd = pos[dst] - pos[src] from gathered columns;
                        # transpose [128,2] -> [2,128] into the chunk pd tile
                        posd_s = eb.tile([128, cfg.TB, DIM], f32, tag="posd")
                        nc.vector.tensor_tensor(
                            posd_s[:, :, :], ug_b[:, :, AH:UW],
                            vhg_b[:, :, AH + C:VW], op=ALU.subtract)
                        for tt in range(cfg.TB):
                            ti = b * cfg.TB + tt
                            pdT_p = ps_t.tile([128, 128], f32, tag="tr")
                            nc.tensor.transpose(pdT_p[:DIM, :], posd_s[:, tt, :],
                                                ident_s[:])
                            nc.scalar.activation(
                                pd_s[:, ti * 128:(ti + 1) * 128],
                                pdT_p[:DIM, :], AF.Copy)

                        # pos MLP
                        tp1_p = ps_m.tile([PH, BLK], f32, tag="tp1")
                        nc.tensor.matmul(tp1_p[:], lhsT=wp1_m[:],
                                         rhs=pd_s[:, esl], start=True, stop=True)
                        tp1_s = eb.tile([PH, BLK], mdt, tag="tp1s")
                        nc.scalar.activation(tp1_s[:], tp1_p[:], AF.Relu, bias=bias_s[0:PH, 0:1])
                        del_p = ps_b.tile([C, BLK], f32, tag="delp")
                        nc.tensor.matmul(del_p[:], lhsT=wp2_m[:],
                                         rhs=tp1_s[:], start=True, stop=True)
                        del_s = eb.tile([C, BLK], f32, tag="dels")
                        nc.scalar.activation(del_s[:], del_p[:], AF.Relu, bias=bias_s[:, 1:2])
                        if mdt is f32r:
                            del_m = eb.tile([C, BLK], f32r, tag="delm")
                            nc.scalar.activation(del_m[:], del_p[:], AF.Relu, bias=bias_s[:, 1:2])
                        else:
                            del_m = del_s

                        # attn layer 1: z1 = Wa1^T delta ; t_a = relu(z1 + gd^T + ba1)
                        z1_p = ps_n.tile([AH, BLK], f32, tag="z1")
                        nc.tensor.matmul(z1_p[:], lhsT=wa1_m[:],
                                         rhs=del_m[:], start=True, stop=True)
                        gd_b = eb.tile([128, cfg.TB, AH], f32, tag="gd")
                        nc.vector.tensor_tensor(gd_b[:, :, :], ug_b[:, :, 0:AH],
                                                vhg_b[:, :, 0:AH],
                                                op=ALU.subtract)
                        gdT_s = eb.tile([AH, BLK], f32, tag="gdT")
                        for tt in range(cfg.TB):
                            gdT_p = ps_t.tile([128, 128], f32, tag="tr")
                            nc.tensor.transpose(gdT_p[:AH, :], gd_b[:, tt, :],
                                                ident_s[:])
                            csl = slice(tt * 128, (tt + 1) * 128)
                            nc.scalar.activation(gdT_s[:, csl], gdT_p[:AH, :], AF.Copy)
                        tsum_s = eb.tile([AH, BLK], f32, tag="tsum")
                        nc.vector.tensor_tensor(tsum_s[:], z1_p[:], gdT_s[:],
                                                op=ALU.add)
                        ta_s = eb.tile([AH, BLK], mdt, tag="ta")
                        nc.scalar.activation(ta_s[:], tsum_s[:], AF.Relu, bias=bias_s[0:AH, 2:3])

                        # attn layer 2 + exp
                        al_p = ps_c.tile([C, BLK], f32, tag="al")
                        nc.tensor.matmul(al_p[:], lhsT=wa2_m[:],
                                         rhs=ta_s[:], start=True, stop=True)
                        ar_s = eb.tile([C, BLK], f32, tag="ar")
                        nc.scalar.activation(ar_s[:], al_p[:], AF.Relu, bias=bias_s[:, 3:4])
                        e_s = eb.tile([C, BLK], f32, tag="e")
                        nc.scalar.activation(e_s[:], ar_s[:], AF.Exp, bias=bias_s[:, 4:5])
                        ew2_s = eb.tile([C, BLK], f32, tag="ew2")
                        nc.vector.tensor_tensor(ew2_s[:], e_s[:], del_s[:], op=ALU.mult)
                        del del_s  # f32 copy only feeds ew2

                        # per-tile: transpose, assemble [ew | e]^T, indicator, seg-matmul
                        for tt in range(cfg.TB):
                            ti = b * cfg.TB + tt
                            csl = slice(tt * 128, (tt + 1) * 128)
                            eT_p = ps_t.tile([128, 128], f32, tag="tr")
                            nc.tensor.transpose(eT_p[:], e_s[:, csl], ident_s[:])
                            ew2T_p = ps_t.tile([128, 128], f32, tag="tr")
                            nc.tensor.transpose(ew2T_p[:], ew2_s[:, csl], ident_s[:])
                            ewe_s = eb.tile([128, 2 * C], mdt, tag="ewe")
                            nc.vector.tensor_copy(ewe_s[:, C:], eT_p[:])
                            tmp_s = eb.tile([128, C], f32, tag="tmp")
                            nc.vector.tensor_tensor(tmp_s[:], eT_p[:],
                                                    vhgs[tt][:, AH:AH + C],
                                                    op=ALU.mult)
                            nc.vector.tensor_tensor(ewe_s[:, 0:C], tmp_s[:], ew2T_p[:],
                                                    op=ALU.add)
                            ind_s = eb.tile([128, 128], mdt, tag="ind")
                            nc.vector.tensor_scalar(ind_s[:], iota_s[:], dl_s[:, ti:ti + 1],
                                                    None, op0=ALU.is_equal)
                            nc.tensor.matmul(acc_p[:], lhsT=ind_s[:],
                                             rhs=ewe_s[:],
                                             start=(ti == 0), stop=(ti == cfg.T - 1))

                    # finalize chunk
                    sp_s = eb.tile([128, C], f32, tag="sp")
                    nc.vector.tensor_scalar_add(sp_s[:], acc_p[:, C:], cfg.EPS)
                    rp_s = eb.tile([128, C], f32, tag="rp")
                    nc.vector.reciprocal(rp_s[:], sp_s[:])
                    o_s = eb.tile([128, C], f32, tag="o")
                    nc.vector.tensor_tensor(o_s[:], acc_p[:, 0:C], rp_s[:], op=ALU.mult)
                    o2_s = eb.tile([128, C], f16, tag="o2")
                    nc.scalar.activation(o2_s[:], o_s[:], AF.Relu)
                    nc.gpsimd.indirect_dma_start(
                        out=y_d[:], out_offset=IndirectOffsetOnAxis(ap=or_s[:, :1], axis=0),
                        in_=o2_s[:], in_offset=None)
    nc.finalize()
    return nc


def _build_inputs(inputs, cfg):
    x = np.ascontiguousarray(np.asarray(inputs["x"], np.float32))
    pos = np.ascontiguousarray(np.asarray(inputs["pos"], np.float32))
    W_lin = np.asarray(inputs["W_lin"], np.float32)
    W_src = np.asarray(inputs["W_src"], np.float32)
    W_dst = np.asarray(inputs["W_dst"], np.float32)
    Wp1 = np.asarray(inputs["Wp1"], np.float32)
    bp1 = np.asarray(inputs["bp1"], np.float32)
    Wp2 = np.asarray(inputs["Wp2"], np.float32)
    bp2 = np.asarray(inputs["bp2"], np.float32)
    Wa1 = np.asarray(inputs["Wa1"], np.float32)
    ba1 = np.asarray(inputs["ba1"], np.float32)
    Wa2 = np.asarray(inputs["Wa2"], np.float32)
    ba2 = np.asarray(inputs["ba2"], np.float32)

    Wda = (W_dst @ Wa1).astype(np.float32)   # [C, AH]
    Wsa = (W_src @ Wa1).astype(np.float32)
    wnode = np.concatenate([Wda, Wsa, W_lin], axis=1)  # [C, 2AH + C]
    bias = np.zeros((128, 5), np.float32)
    bias[: cfg.PH, 0] = bp1
    bias[: cfg.C, 1] = bp2
    bias[: cfg.AH, 2] = ba1
    bias[: cfg.C, 3] = ba2
    bias[:, 4] = -cfg.SHIFT

    packs, nchunk = _pack(x, pos, inputs["edge_index"], cfg)
    common = dict(Wnode=np.ascontiguousarray(wnode),
                  Wp1=np.ascontiguousarray(Wp1), Wp2=np.ascontiguousarray(Wp2),
                  Wa2=np.ascontiguousarray(Wa2), bias=bias)
    common["Wa1p"] = np.ascontiguousarray(Wa1)
    NLOC = cfg.NLOC
    xsc = np.maximum(np.abs(x).max(axis=1, keepdims=True), 1e-30) / 127.0
    xq = np.round(x / xsc).astype(np.int8)
    in_maps = [
        dict(common,
             x=np.ascontiguousarray(xq[c * NLOC:(c + 1) * NLOC]),
             xscale=np.ascontiguousarray(xsc[c * NLOC:(c + 1) * NLOC]),
             pos=np.ascontiguousarray(pos[c * NLOC:(c + 1) * NLOC]),
             **p)
        for c, p in enumerate(packs)
    ]
    return in_maps, nchunk


def kernel(**inputs):
    cfg = CFG
    in_maps, nchunk = _build_inputs(inputs, cfg)
    nc = _build(cfg, nchunk)
    res = run_bass_kernel_spmd(nc, in_maps, list(range(cfg.M)))
    y = np.concatenate(
        [res.results[c]["y"][: cfg.NLOC].astype(np.float32) for c in range(cfg.M)],
        axis=0,
    )
    return y
